# revision 35
# baseline (speedup 1.0000x reference)
"""Multi-head attention Trainium2 kernel (B=2, S=2048, D=1024, H=16, Dh=64).

Sharding: 8 cores = 2 (batch) x 4 (head-groups of 4 heads).
Each core computes qT/kT/v projections for its 4 heads, attention, and a
partial (row-sharded) output projection. Host sums the 4 head-group
partials per batch and adds bo.

v2 layout: per query-block of 512, per head-pair, scores for both heads of
the pair go into one [128,2,512] psum (row-tiled matmuls), one ACT exp per
k-tile covers both heads, and the attn@V matmuls are column-tiled so both
heads run concurrently in the PE array (head A -> psum partitions 0:64,
head B -> 64:128). Softmax denominators come from a 4-way column-tiled
M=1 ones-matmul accumulated over k-tiles; reciprocals via the fast DVE
approximation; normalization lands directly in the Wo-ready layout.
Projection / output-projection matmuls are interleaved into the attention
emission so the PE fills the slack while ACT (exp) runs saturated.
"""

import sys

sys.path.insert(0, "/opt/trn_rl_repo")

import ml_dtypes
import numpy as np

import concourse.bass as bass  # noqa: F401
import concourse.mybir as mybir
import concourse.tile as tile
from concourse import bacc, bass_utils

F32 = mybir.dt.float32
F32R = mybir.dt.float32r
BF16 = mybir.dt.bfloat16
F8 = mybir.dt.float8e4
AF = mybir.ActivationFunctionType
DR = mybir.MatmulPerfMode.DoubleRow

B, S, D = 2, 2048, 1024
H, DH = 16, 64
N_CORES = 8
HPC = 4  # heads per core
CW = HPC * DH  # c-width per core (256)
SBLK = 512  # s_q block size
NSBLK = S // SBLK  # 4
NKT = S // 128  # 16 s_k tiles
KD = D // 128  # 8 contraction tiles for projections

_CACHE = {}


def _build_program():
    nc = bacc.Bacc("TRN2", target_bir_lowering=False, debug=False, num_devices=N_CORES)

    qT_d = nc.dram_tensor("qT", [D, S], BF16, kind="ExternalInput").ap()
    kTx_d = nc.dram_tensor("kTx", [D, S], BF16, kind="ExternalInput").ap()
    vTx_d = nc.dram_tensor("vTx", [D, S], BF16, kind="ExternalInput").ap()
    wq_d = nc.dram_tensor("wqT", [D, CW], BF16, kind="ExternalInput").ap()
    wk_d = nc.dram_tensor("wkT", [D, CW], BF16, kind="ExternalInput").ap()
    wv_d = nc.dram_tensor("wvT", [D, CW], BF16, kind="ExternalInput").ap()
    wo_d = nc.dram_tensor("woT", [CW, D], BF16, kind="ExternalInput").ap()
    bq_d = nc.dram_tensor("bq", [CW], F32, kind="ExternalInput").ap()
    bk_d = nc.dram_tensor("bk", [CW], F32, kind="ExternalInput").ap()
    bv_d = nc.dram_tensor("bv", [CW], F32, kind="ExternalInput").ap()
    out_d = nc.dram_tensor("outT", [D, S], F32, kind="ExternalOutput").ap()

    with tile.TileContext(nc) as tc:
        _kernel_body(nc, tc, qT_d, kTx_d, vTx_d, wq_d, wk_d, wv_d, wo_d,
                     bq_d, bk_d, bv_d, out_d)
    nc.compile()
    return nc


def _kernel_body(nc, tc, qT_d, kTx_d, vTx_d, wq_d, wk_d, wv_d, wo_d,
                 bq_d, bk_d, bv_d, out_d):
    from contextlib import ExitStack

    SCALE = float(1.0 / np.sqrt(DH))

    ctx = ExitStack()
    with ctx:
        const = ctx.enter_context(tc.tile_pool(name="const", bufs=1))
        persist = ctx.enter_context(tc.tile_pool(name="persist", bufs=1))
        xio = ctx.enter_context(tc.tile_pool(name="xio", bufs=1))
        strip_pool = ctx.enter_context(tc.tile_pool(name="strip", bufs=3))
        smp = ctx.enter_context(tc.tile_pool(name="sm", bufs=2))
        outp = ctx.enter_context(tc.tile_pool(name="outp", bufs=2))
        scps = ctx.enter_context(tc.tile_pool(name="scps", bufs=2, space="PSUM"))
        avps = ctx.enter_context(tc.tile_pool(name="avps", bufs=1, space="PSUM"))
        pjps = ctx.enter_context(tc.tile_pool(name="pjps", bufs=2, space="PSUM"))

        # ---- weights / biases ----
        wq_sb = const.tile([128, KD, CW], BF16, tag="wq")
        wk_sb = const.tile([128, KD, CW], BF16, tag="wk")
        wv_sb = const.tile([128, KD, CW], BF16, tag="wv")
        wo_sb = const.tile([128, CW // 128, D], BF16, tag="wo")
        bq_sb = const.tile([128, 2], F32, tag="bq")
        bk_sb = const.tile([128, 2], F32, tag="bk")
        # Prologue loads are spread across engine DGE queues so the ~50KB/
        # partition of inputs+weights transfer in parallel instead of
        # serializing behind one queue (sync carries xk0 in the schedule
        # section; scalar takes the K/Q weights, vector xq0, gpsimd vx0+wv).
        nc.scalar.dma_start(out=wk_sb, in_=wk_d.rearrange("(k p) c -> p k c", p=128))
        for p in range(2):
            nc.scalar.dma_start(out=bk_sb[:, p : p + 1],
                                in_=bk_d[p * 128 : (p + 1) * 128].unsqueeze(1))
        nc.scalar.dma_start(out=wq_sb, in_=wq_d.rearrange("(k p) c -> p k c", p=128))
        for p in range(2):
            nc.scalar.dma_start(out=bq_sb[:, p : p + 1],
                                in_=bq_d[p * 128 : (p + 1) * 128].unsqueeze(1))
        nc.gpsimd.dma_start(out=wv_sb, in_=wv_d.rearrange("(k p) c -> p k c", p=128))
        nc.scalar.dma_start(out=wo_sb, in_=wo_d.rearrange("(ct p) e -> p ct e", p=128))
        bv_row = const.tile([1, CW], F32, tag="bvr")
        nc.gpsimd.dma_start(out=bv_row, in_=bv_d.unsqueeze(0))
        bv_bc = const.tile([128, CW], F32, tag="bvb")
        nc.gpsimd.partition_broadcast(bv_bc, bv_row)


        # ---- persistent activations ----
        qT_sb = [persist.tile([128, S], BF16, tag=f"qT{p}", name=f"qT_sb{p}") for p in range(2)]
        kT_sb = [persist.tile([128, S], BF16, tag=f"kT{p}", name=f"kT_sb{p}") for p in range(2)]
        v_sb = persist.tile([128, NKT, HPC, DH + 1], BF16, tag="v")
        nc.vector.memset(v_sb[:, :, :, DH : DH + 1], 1.0)
        ao_sb = persist.tile([128, CW // 128, S], BF16, tag="ao")

        qTr = qT_d.rearrange("(k p) s -> p k s", p=128)
        kTr = kTx_d.rearrange("(k p) s -> p k s", p=128)
        vTr = vTx_d.rearrange("(k p) s -> p k s", p=128)

        # ---------------- input loads ----------------
        xk_t, xq_t, vx_t = {}, {}, {}

        def dma_xk(sb):
            # xk bufs: 4 live tiles (no recycling -> no WAR hazards)
            t = xio.tile([128, KD, SBLK], BF16, tag=f"xk{sb}", name=f"xk{sb}")
            nc.sync.dma_start(out=t, in_=kTr[:, :, sb * SBLK : (sb + 1) * SBLK])
            xk_t[sb] = t

        def dma_xq(sb):
            t = xio.tile([128, KD, SBLK], BF16, tag=f"xq{sb % 2}", name=f"xq{sb}")
            eng = nc.scalar if sb == 0 else nc.sync
            eng.dma_start(out=t, in_=qTr[:, :, sb * SBLK : (sb + 1) * SBLK])
            xq_t[sb] = t

        def dma_vx(q):
            t = xio.tile([128, KD, 512], BF16, tag=f"vx{q % 2}", name=f"vx{q}")
            eng = nc.gpsimd if q == 0 else nc.sync
            eng.dma_start(out=t, in_=vTr[:, :, q * 512 : (q + 1) * 512])
            vx_t[q] = t

        # ---------------- projection units ----------------
        def proj_kq(which, sb, p):
            """One pair's K or Q projection for s-block sb (8 matmuls N=512)."""
            x = (xk_t if which == "k" else xq_t)[sb]
            w = wk_sb if which == "k" else wq_sb
            dest = (kT_sb if which == "k" else qT_sb)[p]
            b = bk_sb if which == "k" else bq_sb
            ps = pjps.tile([128, SBLK], F32, tag="pj", name=f"ps{which}{sb}{p}")
            for k in range(KD):
                nc.tensor.matmul(ps, w[:, k, p * 128 : (p + 1) * 128],
                                 x[:, k, :], start=(k == 0), stop=(k == KD - 1))
            nc.vector.tensor_scalar_add(dest[:, sb * SBLK : (sb + 1) * SBLK],
                                        ps, b[:, p : p + 1])

        def proj_v(t):
            """V projection for s_k tile t, all 4 heads (8 matmuls N=256)."""
            q, ti = divmod(t, 4)
            ps = pjps.tile([128, SBLK], F32, tag="pj", name=f"psv{t}")
            psv = ps[:, 0:CW]
            for k in range(KD):
                nc.tensor.matmul(psv, vx_t[q][:, k, ti * 128 : (ti + 1) * 128],
                                 wv_sb[:, k, :], start=(k == 0), stop=(k == KD - 1))
            nc.vector.tensor_add(
                v_sb[:, t, :, 0:DH],
                psv.rearrange("p (h c) -> p h c", h=HPC),
                bv_bc.rearrange("p (h c) -> p h c", h=HPC),
            )

        def outproj_unit(blk, et):
            sq = slice(blk * SBLK, (blk + 1) * SBLK)
            ps = pjps.tile([128, SBLK], F32, tag="pj", name=f"pso{blk}{et}")
            for ct in range(CW // 128):
                nc.tensor.matmul(ps, wo_sb[:, ct, et * 128 : (et + 1) * 128],
                                 ao_sb[:, ct, sq], start=(ct == 0),
                                 stop=(ct == CW // 128 - 1))
            osb = outp.tile([128, SBLK], F32, tag="osb", name=f"osb{blk}{et}")
            nc.vector.tensor_copy(osb, ps)
            nc.sync.dma_start(
                out=out_d.rearrange("(et p) s -> p et s", p=128)[:, et, sq],
                in_=osb)

        # ---------------- attention ----------------
        def attn_pair(blk, p, fillers):
            """One head-pair's scores/exp/attn@V over all 16 k-tiles for
            query-block blk. fillers: {t: [callables]} emitted before tile t.
            V carries a fused ones-column, so row 64 of each av psum is the
            softmax denominator."""
            sq = slice(blk * SBLK, (blk + 1) * SBLK)
            avA = avps.tile([DH + 1, SBLK], F32, tag="avA", name=f"avA{blk}{p}")
            avB = avps.tile([DH + 1, SBLK], F32, tag="avB", name=f"avB{blk}{p}")
            def emit_av(t, strip, tl):
                nc.tensor.matmul(avA, v_sb[:, t, 2 * p, :],
                                 strip[:, tl, 0, :],
                                 start=(t == 0), stop=(t == NKT - 1))
                nc.tensor.matmul(avB, v_sb[:, t, 2 * p + 1, :],
                                 strip[:, tl, 1, :],
                                 start=(t == 0), stop=(t == NKT - 1))

            # Software pipeline: av(t) is emitted after sc(t+1)/exp(t+1), so
            # the PE queue head never blocks on exp(t) with ready sc work
            # stuck behind it.
            pend = None
            for half in range(2):
                strip = strip_pool.tile([128, 8, 2, SBLK], BF16, tag="strip",
                                        name=f"strip{blk}{p}{half}")
                for tl in range(8):
                    t = half * 8 + tl
                    for f in fillers.get(t, ()):
                        f()
                    sc = scps.tile([128, 2, SBLK], F32, tag="sc",
                                   name=f"sc{blk}{p}{t}")
                    nc.tensor.matmul(sc[:, 0, :],
                                     kT_sb[p][0:64, t * 128 : (t + 1) * 128],
                                     qT_sb[p][0:64, sq], start=True, stop=True)
                    nc.tensor.matmul(sc[:, 1, :],
                                     kT_sb[p][64:128, t * 128 : (t + 1) * 128],
                                     qT_sb[p][64:128, sq], start=True, stop=True)
                    nc.scalar.activation(strip[:, tl, :, :], sc, AF.Exp,
                                         scale=SCALE)
                    if pend is not None:
                        emit_av(*pend)
                    pend = (t, strip, tl)
            emit_av(*pend)
            return avA, avB

        def normalize(blk, p, avA, avB):
            # Denominators sit on partition 64 of each av psum. DVE copy them
            # (lane-aligned) into a staging tile, DMA each row to a fresh
            # partition-0 [1,SBLK] tile (partition_broadcast reads tile
            # origin only), reciprocal there, broadcast per head, multiply.
            # Head A lands in ao rows 0:64 directly; head B is multiplied in
            # lanes 0:64 then DMA-moved to ao rows 64:128.
            sq = slice(blk * SBLK, (blk + 1) * SBLK)
            stage = smp.tile([128, 2, SBLK], F32, tag="stg",
                             name=f"stg{blk}{p}")
            nc.vector.tensor_copy(stage[64:65, 0, :], avA[DH : DH + 1, :])
            nc.vector.tensor_copy(stage[64:65, 1, :], avB[DH : DH + 1, :])
            rhs_ = []
            for j in range(2):
                rh = smp.tile([1, SBLK], F32, tag="rh", name=f"rh{blk}{p}{j}",
                              bufs=4)
                nc.sync.dma_start(out=rh, in_=stage[64:65, j, :])
                rc = smp.tile([1, SBLK], F32, tag="rc", name=f"rc{blk}{p}{j}",
                              bufs=4)
                nc.vector.reciprocal_approx_fast(rc, rh)
                rb = smp.tile([64, SBLK], F32, tag="rb", name=f"rb{blk}{p}{j}",
                              bufs=4)
                nc.gpsimd.partition_broadcast(rb, rc)
                rhs_.append(rb)
            nc.vector.tensor_mul(ao_sb[0:64, p, sq], avA[0:DH, :], rhs_[0])
            stgB = smp.tile([64, SBLK], BF16, tag="stgB", name=f"stgB{blk}{p}")
            nc.vector.tensor_mul(stgB, avB[0:DH, :], rhs_[1])
            nc.sync.dma_start(out=ao_sb[64:128, p, sq], in_=stgB)

        # ---------------- emission schedule ----------------
        # Prologue: first input loads + enough K/Q for blk0 pair0.
        dma_xk(0)
        dma_xq(0)
        dma_vx(0)
        proj_kq("k", 0, 0)
        proj_kq("k", 0, 1)
        proj_kq("q", 0, 0)

        for blk in range(NSBLK):
            # pair-0 fillers
            f0 = {}
            if blk == 0:
                # V projection per k-tile (needed by av(t) of this very loop),
                # remaining K projections for pair 0, input DMAs.
                for t in range(NKT):
                    f0.setdefault(t, []).append(lambda t=t: proj_v(t))
                f0.setdefault(0, []).insert(0, lambda: dma_vx(1))
                f0.setdefault(1, []).insert(0, lambda: dma_xk(1))
                f0.setdefault(2, []).append(lambda: proj_kq("k", 1, 0))
                f0.setdefault(4, []).insert(0, lambda: dma_vx(2))
                f0.setdefault(5, []).insert(0, lambda: dma_xk(2))
                f0.setdefault(6, []).append(lambda: proj_kq("k", 2, 0))
                f0.setdefault(8, []).insert(0, lambda: dma_vx(3))
                f0.setdefault(8, []).append(lambda: dma_xk(3))
                f0.setdefault(9, []).append(lambda: proj_kq("k", 3, 0))
                f0.setdefault(13, []).append(lambda: proj_kq("q", 0, 1))
            else:
                # output projection of the previous block (first half) and
                # the Q projection needed by this block's pair 1.
                prev = blk - 1
                for i in range(4):
                    f0.setdefault(2 + 3 * i, []).append(
                        lambda prev=prev, i=i: outproj_unit(prev, i))
                f0.setdefault(6, []).append(
                    lambda blk=blk: proj_kq("q", blk, 1))
                if blk < NSBLK - 1:
                    f0.setdefault(10, []).append(
                        lambda blk=blk: dma_xq(blk + 1))

            avA0, avB0 = attn_pair(blk, 0, f0)
            normalize(blk, 0, avA0, avB0)

            # pair-1 fillers
            f1 = {}
            if blk == 0:
                f1.setdefault(0, []).append(lambda: proj_kq("k", 1, 1))
                f1.setdefault(4, []).append(lambda: proj_kq("k", 2, 1))
                f1.setdefault(8, []).append(lambda: proj_kq("k", 3, 1))
                f1.setdefault(12, []).append(lambda: dma_xq(1))
                f1.setdefault(13, []).append(lambda: proj_kq("q", 1, 0))
            else:
                prev = blk - 1
                for i in range(4):
                    f1.setdefault(2 + 3 * i, []).append(
                        lambda prev=prev, i=i: outproj_unit(prev, 4 + i))
                if blk < NSBLK - 1:
                    f1.setdefault(6, []).append(
                        lambda blk=blk: proj_kq("q", blk + 1, 0))

            avA1, avB1 = attn_pair(blk, 1, f1)
            normalize(blk, 1, avA1, avB1)

        # tail: final block's output projection
        for et in range(KD):
            outproj_unit(NSBLK - 1, et)


def _get_program():
    if "nc" not in _CACHE:
        _CACHE["nc"] = _build_program()
    return _CACHE["nc"]


LAST_RESULTS = None


def kernel(query, key_, value, Wq, bq, Wk, bk, Wv, bv, Wo, bo):
    global LAST_RESULTS
    query = np.asarray(query, dtype=np.float32)
    key_ = np.asarray(key_, dtype=np.float32)
    value = np.asarray(value, dtype=np.float32)
    Wq = np.asarray(Wq, dtype=np.float32)
    Wk = np.asarray(Wk, dtype=np.float32)
    Wv = np.asarray(Wv, dtype=np.float32)
    Wo = np.asarray(Wo, dtype=np.float32)
    bq = np.asarray(bq, dtype=np.float32)
    bk = np.asarray(bk, dtype=np.float32)
    bv = np.asarray(bv, dtype=np.float32)
    bo = np.asarray(bo, dtype=np.float32)

    nc = _get_program()

    BF = ml_dtypes.bfloat16
    qT = [np.ascontiguousarray(query[b].T.astype(BF)) for b in range(B)]
    kT = [np.ascontiguousarray(key_[b].T.astype(BF)) for b in range(B)]
    vT = [np.ascontiguousarray(value[b].T.astype(BF)) for b in range(B)]

    in_maps = []
    for c in range(N_CORES):
        b, hp = divmod(c, HPC)
        cs = slice(hp * CW, (hp + 1) * CW)
        in_maps.append({
            "qT": qT[b], "kTx": kT[b], "vTx": vT[b],
            "wqT": np.ascontiguousarray(Wq[cs, :].T.astype(BF)),
            "wkT": np.ascontiguousarray(Wk[cs, :].T.astype(BF)),
            "wvT": np.ascontiguousarray(Wv[cs, :].T.astype(BF)),
            "woT": np.ascontiguousarray(Wo[:, cs].T.astype(BF)),
            "bq": np.ascontiguousarray(bq[cs]),
            "bk": np.ascontiguousarray(bk[cs]),
            "bv": np.ascontiguousarray(bv[cs]),
        })

    res = bass_utils.run_bass_kernel_spmd(nc, in_maps, core_ids=list(range(N_CORES)))
    LAST_RESULTS = res

    out = np.zeros((B, S, D), dtype=np.float32)
    for c in range(N_CORES):
        b = c // HPC
        out[b] += res.results[c]["outT"].T
    out += bo
    return out
